# revision 44
# baseline (speedup 1.0000x reference)
"""Multi-head causal attention with RoPE on 8 Trainium2 NeuronCores.

Problem: B=2, S=2048, D=1024, H=16 heads (dk=64), fp32 in/out, causal mask,
RoPE on Q/K, y = softmax(QK^T/sqrt(dk)) V projected by Wo.

Sharding: head-parallel compute, token-parallel output. Core c owns 2 heads
(columns c*128:(c+1)*128 of the QKV projection output) and, for the output
projection, tokens [c*256,(c+1)*256) of each batch. Per core:
  1. Q^T,K^T,V^T for its heads from the full x in bf16 (K-dim 1024 matmuls),
  2. RoPE in the transposed [head_dim, token] layout (pair-swap via two
     strided DMAs straight out of PSUM),
  3. V^T flipped to [token, feature] via 16-bit DMA transposes; a ones
     column appended per head makes the PV matmul emit the softmax
     denominator,
  4. causal attention with transposed scores ST[k,q]: both heads' scores
     land in one 2-bank PSUM tile, a single fused exp covers both, diagonal
     tiles shrink to the valid column range with a [128,128] triangular
     bf16 mask on DVE,
  5. normalization: fast DVE reciprocal of the denominator row, broadcast
     across 64 partitions by a K=1 ones matmul,
  6. per-batch AllToAll (bf16) flips head-sharded -> token-sharded; batch
     0's collective is issued before batch 1's compute so it is hidden,
  7. output projection for the core's 2x256 tokens.
A tiny dummy AllToAll at kernel start absorbs the one-time collective
setup cost (~90us) under the projection phase.
"""

import sys

for p in ("/opt/trn_rl_repo", "/root/.axon_site/_ro/trn_rl_repo"):
    if p not in sys.path:
        sys.path.insert(0, p)

import math

import numpy as np
import ml_dtypes

import os

import concourse.bass as bass
import concourse.tile as tile
from concourse import mybir
from concourse.bass_utils import run_bass_kernel_spmd

DEBUG_DUMP = bool(os.environ.get("KDBG"))

N_CORES = 8
B, S, D, H = 2, 2048, 1024, 16
DK = D // H          # 64
HPC = H // N_CORES   # heads per core = 2
FW = HPC * DK        # head-group width per core = 128
T = B * S            # 4096 flattened tokens
TCH = 512            # token chunk for projections
NCH = T // TCH       # 8 chunks
KT = 128             # k tile
QC = 512             # q chunk in attention
TPC = S // N_CORES   # 256 tokens per core per batch (output ownership)

F32 = mybir.dt.float32
F32R = mybir.dt.float32r
BF16 = mybir.dt.bfloat16


def _spill_waits(nc, max_other=1):
    """walrus in this container allows 1 sync-wait per instruction; move
    excess waits onto preceding single-wait NoOps on the same engine."""
    n_new = 0
    for bb in nc.m.functions[0].blocks:
        newlist = []
        changed = False
        for inst in bb.instructions:
            si = inst.sync_info
            if si is not None and si.on_wait and len(si.on_wait) > max_other:
                waits = list(si.on_wait)
                overflow, keep = waits[:-max_other], waits[-max_other:]
                while overflow:
                    chunk, overflow = overflow[:1], overflow[1:]
                    nop = mybir.InstNoOp(
                        name=f"waitspill{n_new}-{inst.name}", ins=[], outs=[]
                    )
                    nop.engine = inst.engine
                    nop.debug = inst.debug
                    nop.sync_info = mybir.SyncInfo(on_wait=chunk, on_update=[])
                    newlist.append(nop)
                    n_new += 1
                si.on_wait = keep
                inst.sync_info = si
                changed = True
            newlist.append(inst)
        if changed:
            bb.instructions = newlist
    return n_new


def build_kernel():
    nc = bass.Bass("TRN2", num_devices=N_CORES)

    xT = nc.dram_tensor("xT", [D, T], BF16, kind="ExternalInput")
    wqkv = nc.dram_tensor("wqkv", [128, 8 * 3 * FW], BF16, kind="ExternalInput")
    woT = nc.dram_tensor("woT", [D, D], BF16, kind="ExternalInput")
    ctab = nc.dram_tensor("ctab", [FW, S], F32, kind="ExternalInput")
    stab = nc.dram_tensor("stab", [FW, S], F32, kind="ExternalInput")
    trim = nc.dram_tensor("trim", [KT, KT], BF16, kind="ExternalInput")
    y = nc.dram_tensor("y", [2 * TPC, D], F32, kind="ExternalOutput")

    xT_r = xT.rearrange("(dt p) t -> p dt t", p=128)  # [128, 8, T]

    with tile.TileContext(nc) as tc:
        with (
            tc.tile_pool(name="const", bufs=1) as const,
            tc.tile_pool(name="xch", bufs=3) as xch,
            tc.tile_pool(name="qk", bufs=1) as qkpool,
            tc.tile_pool(name="swp", bufs=2) as swpool,
            tc.tile_pool(name="sin", bufs=2) as sinpool,
            tc.tile_pool(name="pts", bufs=4) as pts,
            tc.tile_pool(name="lpool", bufs=2) as lpool,
            tc.tile_pool(name="wo", bufs=8) as wopool,
            tc.tile_pool(name="yout", bufs=2) as ypool,
            tc.tile_pool(name="st", bufs=3, space="PSUM") as stps,
            tc.tile_pool(name="pv", bufs=1, space="PSUM") as pvps,
            tc.tile_pool(name="dram", bufs=1, space="DRAM") as dram,
        ):
            # ---- collective warmup (hidden under projection phase) ----
            warm_in = dram.tile([8, 16], F32)
            warm_out = dram.tile([8, 16], F32)
            wtile = const.tile([1, 128], F32)
            nc.vector.memset(wtile, 0.0)
            nc.gpsimd.dma_start(
                out=warm_in[:, :],
                in_=wtile[:1, :128].rearrange("p (a f) -> (p a) f", a=8),
            )
            nc.gpsimd.collective_compute(
                "AllToAll",
                mybir.AluOpType.bypass,
                replica_groups=[list(range(N_CORES))],
                ins=[warm_in[:].opt()],
                outs=[warm_out[:].opt()],
            )

            # ---- constants (spread across DMA queues; sync is kept free
            # for the x chunk loads so the PE can start ASAP) ----
            w3_sb = const.tile([128, 8, 3, FW], BF16)
            nc.scalar.dma_start(
                out=w3_sb[:, :, :, :].rearrange("p a b f -> p (a b f)"),
                in_=wqkv[:, :],
            )
            wq_sb = w3_sb[:, :, 0, :]
            wk_sb = w3_sb[:, :, 1, :]
            wv_sb = w3_sb[:, :, 2, :]
            c_sb = const.tile([FW, S], F32)
            s_sb = const.tile([FW, S], F32)
            nc.gpsimd.dma_start(out=c_sb, in_=ctab[:, :])
            nc.gpsimd.dma_start(out=s_sb, in_=stab[:, :])
            tri = const.tile([KT, KT], BF16)
            nc.scalar.dma_start(out=tri, in_=trim[:, :])
            ones_f = const.tile([1, DK], F32)
            nc.vector.memset(ones_f, 1.0)
            ones_b = const.tile([1, DK], BF16)
            nc.vector.tensor_copy(out=ones_b, in_=ones_f)

            qT = qkpool.tile([FW, T], BF16, tag="qT")
            kTt = qkpool.tile([FW, T], BF16, tag="kT")
            # V in [token, feature] layout: [part, block, head, dk+1]; the
            # trailing column per head is 1.0 so PV emits the softmax
            # denominator as PSUM row 64, whose banks also host the 1/l
            # broadcast (rows 64:128) after the denominator is drained.
            v4 = qkpool.tile([128, T // 128, HPC, DK + 1], BF16, tag="v")
            outT = qkpool.tile([FW, T], BF16, tag="outT")
            vones = const.tile([128, T // 128], F32)
            nc.vector.memset(vones, 1.0)
            for h2 in range(HPC):
                nc.vector.tensor_copy(out=v4[:, :, h2, DK], in_=vones)

            # ---- QKV projections + RoPE (per x chunk) ----
            xtiles = {}

            def load_x(ci):
                t0 = ci * TCH
                xc = xch.tile([128, 8, TCH], BF16, tag="x", name="xc")
                # two half-loads so the first matmuls can start sooner
                nc.sync.dma_start(out=xc[:, 0:4, :], in_=xT_r[:, 0:4, t0 : t0 + TCH])
                nc.sync.dma_start(out=xc[:, 4:8, :], in_=xT_r[:, 4:8, t0 : t0 + TCH])
                xtiles[ci] = xc

            def do_qkv_chunk(ci):
                t0 = ci * TCH
                sc = (ci % (S // TCH)) * TCH  # position within batch
                xc = xtiles.pop(ci)

                # V first (its PSUM slot drains fastest, keeping the shared
                # pool rotation smooth), directly in [token, feature] layout
                vps2 = stps.tile([128, TCH // 128, HPC, DK], F32, tag="st", name="vps2")
                for sub in range(TCH // 128):
                    for dt in range(8):
                        nc.tensor.matmul(
                            vps2[:, sub, :, :],
                            xc[:, dt, sub * 128 : (sub + 1) * 128],
                            wv_sb[:, dt, :],
                            start=(dt == 0),
                            stop=(dt == 7),
                            skip_group_check=True,
                        )
                idx0 = t0 // 128
                with nc.allow_low_precision(reason="bf16 v"):
                    for h2 in range(HPC):
                        nc.scalar.copy(
                            out=v4[:, idx0 : idx0 + 4, h2, 0:DK],
                            in_=vps2[:, :, h2, :],
                        )

                # Q and K share one 2-bank PSUM slot of the common pool
                qkps = stps.tile([FW, 2, TCH], F32, tag="st", name="qkps")
                for qk in range(2):
                    w_sb = wq_sb if qk == 0 else wk_sb
                    for dt in range(8):
                        nc.tensor.matmul(
                            qkps[:, qk, :],
                            w_sb[:, dt, :],
                            xc[:, dt, :],
                            start=(dt == 0),
                            stop=(dt == 7),
                            skip_group_check=True,
                        )
                for qk, dst in ((0, qT), (1, kTt)):
                    ps = qkps[:, qk, :]
                    # pair swap across partitions via two strided DMAs off a
                    # bf16 staging copy (DMA can't read PSUM directly)
                    raw = swpool.tile([FW, TCH], BF16, tag="raw", name="raw")
                    with nc.allow_low_precision(reason="bf16 rope"):
                        nc.vector.tensor_copy(out=raw, in_=ps)
                    swp = swpool.tile([FW, TCH], BF16, tag="swp", name="swp")
                    eng = nc.scalar if qk == 0 else nc.sync
                    eng.dma_start(out=swp[0 : FW - 1 : 2, :], in_=raw[1:FW:2, :])
                    eng.dma_start(out=swp[1:FW:2, :], in_=raw[0 : FW - 1 : 2, :])
                    dslice = dst[:, t0 : t0 + TCH]
                    with nc.allow_low_precision(reason="bf16 rope"):
                        nc.vector.tensor_mul(dslice, ps, c_sb[:, sc : sc + TCH])
                        t2 = sinpool.tile([FW, TCH], BF16, tag="sin", name="t2")
                        nc.gpsimd.tensor_mul(t2, swp, s_sb[:, sc : sc + TCH])
                        nc.vector.tensor_add(dslice, dslice, t2)

            # ---- causal attention for one batch ----
            # Transposed-scores flash style; both heads share one 2-bank
            # PSUM tile so a single exp serves both. Software-pipelined one
            # k-tile ahead so the PE never waits on exp.
            def emit_st(b, qc, kt):
                trow = b * S + qc * QC
                kcol = b * S + kt * KT
                o = max(0, (kt - 4 * qc) * KT)  # first valid q col (diag tiles)
                st2 = stps.tile([KT, HPC, QC], F32, tag="st", name="st2")
                for h2 in range(HPC):
                    fb = h2 * DK
                    nc.tensor.matmul(
                        st2[:, h2, o:QC],
                        kTt[fb : fb + DK, kcol : kcol + KT],
                        qT[fb : fb + DK, trow + o : trow + QC],
                        start=True,
                        stop=True,
                    )
                pt2 = pts.tile([KT, HPC, QC], BF16, tag="pt", name="pt2")
                with nc.allow_low_precision(reason="bf16 softmax"):
                    nc.scalar.activation(
                        out=pt2[:, :, o:QC],
                        in_=st2[:, :, o:QC],
                        func=mybir.ActivationFunctionType.Exp,
                    )
                    if kt >= 4 * qc:  # diagonal: triangular mask on 128 cols
                        for h2 in range(HPC):
                            nc.vector.tensor_mul(
                                pt2[:, h2, o : o + KT], pt2[:, h2, o : o + KT], tri
                            )
                return pt2, o

            def emit_pv(b, qc, kt, pv2, ptpair):
                pt2, o = ptpair
                nkt = 4 * (qc + 1)
                for h2 in range(HPC):
                    nc.tensor.matmul(
                        pv2[0 : DK + 1, h2, o:QC],
                        v4[:, b * (S // 128) + kt, h2, :],
                        pt2[:, h2, o:QC],
                        start=(kt == 0),
                        stop=(kt == nkt - 1),
                        skip_group_check=True,
                    )

            def do_attn_qc_main(b, qc):
                trow = b * S + qc * QC
                # full-partition pv tile: rows 0:65 = [PV; l], rows 64:128
                # host the 1/l broadcast so no extra PSUM bank is needed
                pv2 = pvps.tile([128, HPC, QC], F32, tag="pv", name="pv2")
                nkt = 4 * (qc + 1)
                window = [emit_st(b, qc, 0)]
                if nkt > 1:
                    window.append(emit_st(b, qc, 1))
                for kt in range(2, nkt):
                    window.append(emit_st(b, qc, kt))
                    emit_pv(b, qc, kt - 2, pv2, window.pop(0))
                for i, ptp in enumerate(window):
                    emit_pv(b, qc, nkt - len(window) + i, pv2, ptp)
                # Normalization, part 1: drain PV to SBUF (frees rows 0:65
                # fast), spread the denominator row across partitions by DMA
                # so the reciprocal is nearly free, gather it back.
                pvc = lpool.tile([DK + 1, HPC, QC], BF16, tag="pvc", name="pvc")
                with nc.allow_low_precision(reason="bf16 pv"):
                    nc.vector.tensor_copy(out=pvc, in_=pv2[0 : DK + 1, :, :])
                nsp = HPC * QC // 128  # 8
                lsp = lpool.tile([128, nsp], BF16, tag="lsp", name="lsp")
                nc.scalar.dma_start(out=lsp, in_=pvc[DK : DK + 1, :, :])
                linv_sp = lpool.tile([128, nsp], BF16, tag="lisp", name="linv_sp")
                with nc.allow_low_precision(reason="bf16 recip"):
                    nc.vector.reciprocal(out=linv_sp, in_=lsp)
                linv2 = lpool.tile([1, HPC, QC], BF16, tag="linv2", name="linv2")
                nc.scalar.dma_start(out=linv2, in_=linv_sp)
                return pv2, pvc, linv2, trow

            def do_attn_qc_norm(state):
                # Normalization, part 2 — emitted AFTER the next chunk's
                # matmuls so the in-order PE/DVE queues never stall on the
                # reciprocal chain: broadcast 1/l via a K=1 ones matmul into
                # rows 64:128 of the pv tile, then scale into outT.
                pv2, pvc, linv2, trow = state
                for h2 in range(HPC):
                    nc.tensor.matmul(
                        pv2[64:128, h2, :],
                        ones_b,
                        linv2[:, h2, :],
                        start=True,
                        stop=True,
                        skip_group_check=True,
                    )
                for h2 in range(HPC):
                    fb = h2 * DK
                    with nc.allow_low_precision(reason="bf16 out"):
                        nc.vector.tensor_mul(
                            outT[fb : fb + DK, trow : trow + QC],
                            pvc[0:DK, h2, :],
                            pv2[64:128, h2, :],
                        )

            # ---- AllToAll staging: head-sharded -> token-sharded ----
            cc_in = []
            cc_out = []
            orecv = []
            for b in range(B):
                cc_in.append(dram.tile([N_CORES, FW, TPC], BF16, name=f"ccin{b}"))
                cc_out.append(dram.tile([N_CORES, FW, TPC], BF16, name=f"ccout{b}"))
                orecv.append(
                    qkpool.tile(
                        [128, N_CORES, TPC], BF16, tag=f"orecv{b}", name=f"orecv{b}"
                    )
                )

            def do_a2a(b):
                nc.gpsimd.dma_start(
                    out=cc_in[b].rearrange("p f t -> f p t"),
                    in_=outT[:, b * S : (b + 1) * S].rearrange(
                        "f (p t) -> f p t", p=N_CORES
                    ),
                )
                nc.gpsimd.collective_compute(
                    "AllToAll",
                    mybir.AluOpType.bypass,
                    replica_groups=[list(range(N_CORES))],
                    ins=[cc_in[b][:].opt()],
                    outs=[cc_out[b][:].opt()],
                )
                nc.gpsimd.dma_start(
                    out=orecv[b][:, :, :],
                    in_=cc_out[b].rearrange("p f t -> f p t"),
                )

            # ---- output projection for one 128-token tile of batch b ----
            def do_proj_tt(b, tt):
                ysb = ypool.tile([128, D], F32, tag="y", name="ysb")
                for ec in range(D // 512):
                    yps = stps.tile([128, 512], F32, tag="st", name="yps")
                    for p in range(N_CORES):
                        nc.tensor.matmul(
                            yps,
                            orecv[b][:, p, tt * 128 : (tt + 1) * 128],
                            wo_sb[p][:, ec * 512 : (ec + 1) * 512],
                            start=(p == 0),
                            stop=(p == N_CORES - 1),
                        )
                    if ec == 0:
                        nc.vector.tensor_copy(
                            out=ysb[:, ec * 512 : (ec + 1) * 512], in_=yps
                        )
                    else:
                        nc.scalar.copy(
                            out=ysb[:, ec * 512 : (ec + 1) * 512], in_=yps
                        )
                nc.sync.dma_start(
                    out=y[b * TPC + tt * 128 : b * TPC + (tt + 1) * 128, :],
                    in_=ysb,
                )

            # schedule: chunk/attention-qc interleave keeps the PE
            # continuously fed (p-state stays at max): attention qc_i of a
            # batch only needs x chunks 0..i of that batch, so each chunk is
            # emitted right before the qc that first needs it, x loads run
            # two segments ahead, and each qc's normalization tail is
            # emitted after the next chunk so no engine queue stalls on the
            # reciprocal chain. The batch-0 AllToAll is kicked mid-stream
            # and hides under batch 1; the batch-0 projection tiles slot
            # into attention-1's tail.
            with nc.named_scope("pipe0"):
                load_x(0)
                load_x(1)
                do_qkv_chunk(0)
                for i in range(S // QC):
                    state = do_attn_qc_main(0, i)
                    load_x(i + 2)
                    if i + 1 < 4:
                        do_qkv_chunk(i + 1)
                    do_attn_qc_norm(state)
            with nc.named_scope("a2a0"):
                do_a2a(0)
            # tables for batch-1 RoPE + projection weights: loaded while
            # attention 0 computes
            wo_sb = []
            for p in range(N_CORES):
                wt = wopool.tile([128, D], BF16, tag="wo", name=f"wo{p}")
                nc.scalar.dma_start(out=wt, in_=woT[p * 128 : (p + 1) * 128, :])
                wo_sb.append(wt)
            with nc.named_scope("pipe1"):
                do_qkv_chunk(4)
                for i in range(S // QC):
                    state = do_attn_qc_main(1, i)
                    if 4 + i + 2 < NCH:
                        load_x(4 + i + 2)
                    if i + 1 < 4:
                        do_qkv_chunk(4 + i + 1)
                    do_attn_qc_norm(state)
            with nc.named_scope("a2a1"):
                do_a2a(1)
            with nc.named_scope("proj"):
                do_proj_tt(0, 0)
                do_proj_tt(0, 1)
                do_proj_tt(1, 0)
                do_proj_tt(1, 1)

            if DEBUG_DUMP:
                dqT = nc.dram_tensor("dqT", [FW, T], BF16, kind="ExternalOutput")
                dkT = nc.dram_tensor("dkT", [FW, T], BF16, kind="ExternalOutput")
                dv4 = nc.dram_tensor(
                    "dv4", [128, T // 128, HPC, DK + 1], BF16, kind="ExternalOutput"
                )
                doutT = nc.dram_tensor("doutT", [FW, T], BF16, kind="ExternalOutput")
                dorecv = nc.dram_tensor(
                    "dorecv", [B, 128, N_CORES, TPC], BF16, kind="ExternalOutput"
                )
                nc.sync.dma_start(out=dqT[:, :], in_=qT)
                nc.sync.dma_start(out=dkT[:, :], in_=kTt)
                nc.sync.dma_start(out=dv4[:, :, :, :], in_=v4)
                nc.sync.dma_start(out=doutT[:, :], in_=outT)
                for b in range(B):
                    nc.sync.dma_start(out=dorecv[b, :, :, :], in_=orecv[b])

    _spill_waits(nc)
    return nc


_NC_CACHE = None


def _get_nc():
    global _NC_CACHE
    if _NC_CACHE is None:
        _NC_CACHE = build_kernel()
    return _NC_CACHE


def _host_prep(x, Wq, Wk, Wv, Wo, token_positions):
    bf = ml_dtypes.bfloat16
    xT = np.ascontiguousarray(x.reshape(T, D).T).astype(bf)  # [D, T]
    WqT = (np.ascontiguousarray(Wq.T) * np.float32(1.0 / math.sqrt(DK))).astype(bf)
    WkT = np.ascontiguousarray(Wk.T).astype(bf)
    WvT = np.ascontiguousarray(Wv.T).astype(bf)
    WoT = np.ascontiguousarray(Wo.T).astype(bf)

    pos = token_positions.astype(np.float64)  # [S]
    i = (np.arange(FW) % DK) // 2  # pair index per row
    inv_freq = 1.0 / (10000.0 ** (2.0 * i / DK))  # [FW]
    ang = inv_freq[:, None] * pos[None, :]  # [FW, S]
    ctab = np.cos(ang).astype(np.float32)
    sgn = np.where(np.arange(FW) % 2 == 0, -1.0, 1.0)
    stab = (np.sin(ang) * sgn[:, None]).astype(np.float32)

    trim = (np.arange(KT)[None, :] >= np.arange(KT)[:, None]).astype(bf)
    return xT, WqT, WkT, WvT, WoT, ctab, stab, trim


def kernel(x, Wq, Wk, Wv, Wo, mask, token_positions, num_heads, **run_kw):
    x = np.asarray(x)
    assert int(num_heads) == H and x.shape == (B, S, D)
    xT, WqT, WkT, WvT, WoT, ctab, stab, trim = _host_prep(
        np.asarray(x, np.float32),
        np.asarray(Wq, np.float32),
        np.asarray(Wk, np.float32),
        np.asarray(Wv, np.float32),
        np.asarray(Wo, np.float32),
        np.asarray(token_positions),
    )
    in_maps = []
    for c in range(N_CORES):
        cols = slice(c * FW, (c + 1) * FW)
        w3 = np.stack([WqT[:, cols], WkT[:, cols], WvT[:, cols]], axis=1)
        wqkv = np.ascontiguousarray(
            w3.reshape(8, 128, 3, FW).transpose(1, 0, 2, 3).reshape(128, 8 * 3 * FW)
        )
        in_maps.append(
            {
                "xT": xT,
                "wqkv": wqkv,
                "woT": WoT,
                "ctab": ctab,
                "stab": stab,
                "trim": trim,
            }
        )
    nc = _get_nc()
    res = run_bass_kernel_spmd(
        nc, in_maps, core_ids=list(range(N_CORES)), **run_kw
    )
    # core c's y rows: [0:TPC] = batch0 tokens [c*TPC,(c+1)*TPC),
    #                  [TPC:2*TPC] = batch1 same range
    out = np.empty((B, S, D), dtype=np.float32)
    for c in range(N_CORES):
        yc = res.results[c]["y"]
        for b in range(B):
            out[b, c * TPC : (c + 1) * TPC, :] = yc[b * TPC : (b + 1) * TPC]
    kernel.last_results = res
    return out


# revision 47
# speedup vs baseline: 1.1427x; 1.1427x over previous
"""Multi-head causal attention with RoPE on 8 Trainium2 NeuronCores.

Problem: B=2, S=2048, D=1024, H=16 heads (dk=64), fp32 in/out, causal mask,
RoPE on Q/K, y = softmax(QK^T/sqrt(dk)) V projected by Wo.

Sharding: head-parallel compute, token-parallel output. Core c owns 2 heads
(columns c*128:(c+1)*128 of the QKV projection output) and, for the output
projection, tokens [c*256,(c+1)*256) of each batch. Per core:
  1. Q^T,K^T,V^T for its heads from the full x in bf16 (K-dim 1024 matmuls),
  2. RoPE in the transposed [head_dim, token] layout (pair-swap via two
     strided DMAs straight out of PSUM),
  3. V computed directly in [token, feature] layout (x-chunk subtiles as
     the stationary operand); a ones column appended per head makes the
     PV matmul emit the softmax denominator,
  4. causal attention with transposed scores ST[k,q]: both heads' scores
     land in one 2-bank PSUM tile, a single fused exp covers both, diagonal
     tiles shrink to the valid column range with a [128,128] triangular
     bf16 mask on DVE,
  5. normalization: fast DVE reciprocal of the denominator row, broadcast
     across 64 partitions by a K=1 ones matmul,
  6. per-batch AllToAll (bf16) flips head-sharded -> token-sharded; batch
     0's collective is issued before batch 1's compute so it is hidden,
  7. output projection for the core's 2x256 tokens.
A tiny dummy AllToAll at kernel start absorbs the one-time collective
setup cost (~90us) under the projection phase.
"""

import sys

for p in ("/opt/trn_rl_repo", "/root/.axon_site/_ro/trn_rl_repo"):
    if p not in sys.path:
        sys.path.insert(0, p)

import math

import numpy as np
import ml_dtypes

import os

import concourse.bass as bass
import concourse.tile as tile
from concourse import mybir
from concourse.bass_utils import run_bass_kernel_spmd

DEBUG_DUMP = bool(os.environ.get("KDBG"))

N_CORES = 8
B, S, D, H = 2, 2048, 1024, 16
DK = D // H          # 64
HPC = H // N_CORES   # heads per core = 2
FW = HPC * DK        # head-group width per core = 128
T = B * S            # 4096 flattened tokens
TCH = 512            # token chunk for projections
NCH = T // TCH       # 8 chunks
KT = 128             # k tile
QC = 512             # q chunk in attention
TPC = S // N_CORES   # 256 tokens per core per batch (output ownership)

F32 = mybir.dt.float32
F32R = mybir.dt.float32r
BF16 = mybir.dt.bfloat16


def _spill_waits(nc, max_other=1):
    """walrus in this container allows 1 sync-wait per instruction; move
    excess waits onto preceding single-wait NoOps on the same engine."""
    n_new = 0
    for bb in nc.m.functions[0].blocks:
        newlist = []
        changed = False
        for inst in bb.instructions:
            si = inst.sync_info
            if si is not None and si.on_wait and len(si.on_wait) > max_other:
                waits = list(si.on_wait)
                overflow, keep = waits[:-max_other], waits[-max_other:]
                while overflow:
                    chunk, overflow = overflow[:1], overflow[1:]
                    nop = mybir.InstNoOp(
                        name=f"waitspill{n_new}-{inst.name}", ins=[], outs=[]
                    )
                    nop.engine = inst.engine
                    nop.debug = inst.debug
                    nop.sync_info = mybir.SyncInfo(on_wait=chunk, on_update=[])
                    newlist.append(nop)
                    n_new += 1
                si.on_wait = keep
                inst.sync_info = si
                changed = True
            newlist.append(inst)
        if changed:
            bb.instructions = newlist
    return n_new


def build_kernel():
    nc = bass.Bass("TRN2", num_devices=N_CORES)

    xT = nc.dram_tensor("xT", [D, T], BF16, kind="ExternalInput")
    wqkv = nc.dram_tensor("wqkv", [128, 8 * 3 * FW], BF16, kind="ExternalInput")
    woT = nc.dram_tensor("woT", [D, D], BF16, kind="ExternalInput")
    ctab = nc.dram_tensor("ctab", [FW, S], F32, kind="ExternalInput")
    stab = nc.dram_tensor("stab", [FW, S], F32, kind="ExternalInput")
    trim = nc.dram_tensor("trim", [KT, KT], BF16, kind="ExternalInput")
    y = nc.dram_tensor("y", [2 * TPC, D], F32, kind="ExternalOutput")

    xT_r = xT.rearrange("(dt p) t -> p dt t", p=128)  # [128, 8, T]

    with tile.TileContext(nc) as tc:
        with (
            tc.tile_pool(name="const", bufs=1) as const,
            tc.tile_pool(name="xch", bufs=2) as xch,
            tc.tile_pool(name="qk", bufs=1) as qkpool,
            tc.tile_pool(name="swp", bufs=2) as swpool,
            tc.tile_pool(name="sin", bufs=2) as sinpool,
            tc.tile_pool(name="vst", bufs=2) as vstpool,
            tc.tile_pool(name="pts", bufs=3) as pts,
            tc.tile_pool(name="lpool", bufs=2) as lpool,
            tc.tile_pool(name="wo", bufs=8) as wopool,
            tc.tile_pool(name="yout", bufs=2) as ypool,
            tc.tile_pool(name="mm", bufs=2, space="PSUM") as mmps,
            tc.tile_pool(name="st", bufs=2, space="PSUM") as stps,
            tc.tile_pool(name="pv", bufs=1, space="PSUM") as pvps,
            tc.tile_pool(name="dram", bufs=1, space="DRAM") as dram,
        ):
            # ---- collective warmup (hidden under projection phase) ----
            warm_in = dram.tile([8, 16], F32)
            warm_out = dram.tile([8, 16], F32)
            wtile = const.tile([1, 128], F32)
            nc.vector.memset(wtile, 0.0)
            nc.gpsimd.dma_start(
                out=warm_in[:, :],
                in_=wtile[:1, :128].rearrange("p (a f) -> (p a) f", a=8),
            )
            nc.gpsimd.collective_compute(
                "AllToAll",
                mybir.AluOpType.bypass,
                replica_groups=[list(range(N_CORES))],
                ins=[warm_in[:].opt()],
                outs=[warm_out[:].opt()],
            )

            # ---- constants ----
            w3_sb = const.tile([128, 8, 3, FW], BF16)
            nc.scalar.dma_start(
                out=w3_sb[:, :, :, :].rearrange("p a b f -> p (a b f)"),
                in_=wqkv[:, :],
            )
            wq_sb = w3_sb[:, :, 0, :]
            wk_sb = w3_sb[:, :, 1, :]
            wv_sb = w3_sb[:, :, 2, :]
            c_sb = const.tile([FW, S], F32)
            s_sb = const.tile([FW, S], F32)
            nc.gpsimd.dma_start(out=c_sb, in_=ctab[:, :])
            nc.gpsimd.dma_start(out=s_sb, in_=stab[:, :])
            tri = const.tile([KT, KT], BF16)
            nc.scalar.dma_start(out=tri, in_=trim[:, :])
            ones_f = const.tile([1, DK], F32)
            nc.vector.memset(ones_f, 1.0)
            ones_b = const.tile([1, DK], BF16)
            nc.vector.tensor_copy(out=ones_b, in_=ones_f)

            qT = qkpool.tile([FW, T], BF16, tag="qT")
            kTt = qkpool.tile([FW, T], BF16, tag="kT")
            # V in [token, feature] layout: [part, block, head, dk+1]; the
            # +1 column per head is 1.0 so PV also emits the denominator.
            v4 = qkpool.tile([128, T // 128, HPC, DK + 1], BF16, tag="v")
            outT = qkpool.tile([FW, T], BF16, tag="outT")
            vones = const.tile([128, T // 128], F32)
            nc.vector.memset(vones, 1.0)
            for h2 in range(HPC):
                nc.vector.tensor_copy(out=v4[:, :, h2, DK], in_=vones)

            # output projection weights (needed last; loaded on scalar queue)
            wo_sb = []
            for p in range(N_CORES):
                wt = wopool.tile([128, D], BF16, tag="wo")
                nc.scalar.dma_start(out=wt, in_=woT[p * 128 : (p + 1) * 128, :])
                wo_sb.append(wt)

            # ---- QKV projections + RoPE (per x chunk) ----
            def do_qkv_chunk(ci):
                t0 = ci * TCH
                sc = (ci % (S // TCH)) * TCH  # position within batch
                xc = xch.tile([128, 8, TCH], BF16, tag="x", name="xc")
                nc.sync.dma_start(out=xc, in_=xT_r[:, :, t0 : t0 + TCH])

                for which, w_sb, dst in (("q", wq_sb, qT), ("k", wk_sb, kTt)):
                    ps = mmps.tile([FW, TCH], F32, tag="mm", name=f"{which}ps")
                    for dt in range(8):
                        nc.tensor.matmul(
                            ps,
                            w_sb[:, dt, :],
                            xc[:, dt, :],
                            start=(dt == 0),
                            stop=(dt == 7),
                        )
                    # pair swap across partitions via two strided DMAs off a
                    # bf16 staging copy (DMA can't read PSUM directly)
                    raw = swpool.tile([FW, TCH], BF16, tag="raw", name="raw")
                    with nc.allow_low_precision(reason="bf16 rope"):
                        nc.scalar.copy(out=raw, in_=ps)
                    swp = swpool.tile([FW, TCH], BF16, tag="swp", name="swp")
                    eng = nc.sync if which == "q" else nc.scalar
                    eng.dma_start(out=swp[0 : FW - 1 : 2, :], in_=raw[1:FW:2, :])
                    eng.dma_start(out=swp[1:FW:2, :], in_=raw[0 : FW - 1 : 2, :])
                    dslice = dst[:, t0 : t0 + TCH]
                    with nc.allow_low_precision(reason="bf16 rope"):
                        nc.vector.tensor_mul(dslice, ps, c_sb[:, sc : sc + TCH])
                        t2 = sinpool.tile([FW, TCH], BF16, tag="sin", name="t2")
                        nc.gpsimd.tensor_mul(t2, swp, s_sb[:, sc : sc + TCH])
                        nc.vector.tensor_add(dslice, dslice, t2)

                # V directly in [token, feature] layout: stationary = x chunk
                # subtiles; all four 128-token subtiles pack into one PSUM bank
                vps2 = mmps.tile([128, TCH // 128, HPC, DK], F32, tag="mm", name="vps2")
                for sub in range(TCH // 128):
                    for dt in range(8):
                        nc.tensor.matmul(
                            vps2[:, sub, :, :],
                            xc[:, dt, sub * 128 : (sub + 1) * 128],
                            wv_sb[:, dt, :],
                            start=(dt == 0),
                            stop=(dt == 7),
                            skip_group_check=True,
                        )
                idx0 = t0 // 128
                with nc.allow_low_precision(reason="bf16 v"):
                    for h2 in range(HPC):
                        nc.scalar.copy(
                            out=v4[:, idx0 : idx0 + 4, h2, 0:DK],
                            in_=vps2[:, :, h2, :],
                        )

            # ---- causal attention for one batch ----
            # Transposed-scores flash style; both heads share one 2-bank
            # PSUM tile so a single exp serves both. Software-pipelined one
            # k-tile ahead so the PE never waits on exp.
            def emit_st(b, qc, kt):
                trow = b * S + qc * QC
                kcol = b * S + kt * KT
                o = max(0, (kt - 4 * qc) * KT)  # first valid q col (diag tiles)
                st2 = stps.tile([KT, HPC, QC], F32, tag="st", name="st2")
                for h2 in range(HPC):
                    fb = h2 * DK
                    nc.tensor.matmul(
                        st2[:, h2, o:QC],
                        kTt[fb : fb + DK, kcol : kcol + KT],
                        qT[fb : fb + DK, trow + o : trow + QC],
                        start=True,
                        stop=True,
                    )
                pt2 = pts.tile([KT, HPC, QC], BF16, tag="pt", name="pt2")
                with nc.allow_low_precision(reason="bf16 softmax"):
                    nc.scalar.activation(
                        out=pt2[:, :, o:QC],
                        in_=st2[:, :, o:QC],
                        func=mybir.ActivationFunctionType.Exp,
                    )
                    if kt >= 4 * qc:  # diagonal: triangular mask on 128 cols
                        for h2 in range(HPC):
                            nc.vector.tensor_mul(
                                pt2[:, h2, o : o + KT], pt2[:, h2, o : o + KT], tri
                            )
                return pt2, o

            def emit_pv(b, qc, kt, pv2, ptpair):
                pt2, o = ptpair
                nkt = 4 * (qc + 1)
                for h2 in range(HPC):
                    nc.tensor.matmul(
                        pv2[:, h2, o:QC],
                        v4[:, b * (S // 128) + kt, h2, :],
                        pt2[:, h2, o:QC],
                        start=(kt == 0),
                        stop=(kt == nkt - 1),
                        skip_group_check=True,
                    )

            def do_attn_batch(b):
                for qc in range(S // QC):
                    trow = b * S + qc * QC
                    pv2 = pvps.tile([DK + 1, HPC, QC], F32, tag="pv", name="pv2")
                    nkt = 4 * (qc + 1)
                    prev = emit_st(b, qc, 0)
                    for kt in range(1, nkt):
                        cur = emit_st(b, qc, kt)
                        emit_pv(b, qc, kt - 1, pv2, prev)
                        prev = cur
                    emit_pv(b, qc, nkt - 1, pv2, prev)
                    # Normalization: drain PV to SBUF (frees the PSUM slot
                    # fast), spread the denominator row across partitions by
                    # DMA so the reciprocal is nearly free, gather it back,
                    # and partition-broadcast on the otherwise-idle gpsimd.
                    pvc = lpool.tile([DK + 1, HPC, QC], BF16, tag="pvc", name="pvc")
                    with nc.allow_low_precision(reason="bf16 pv"):
                        nc.vector.tensor_copy(out=pvc, in_=pv2)
                    nsp = HPC * QC // 128  # 8
                    lsp = lpool.tile([128, nsp], BF16, tag="lsp", name="lsp")
                    nc.sync.dma_start(out=lsp, in_=pvc[DK : DK + 1, :, :])
                    linv_sp = lpool.tile([128, nsp], BF16, tag="lisp", name="linv_sp")
                    with nc.allow_low_precision(reason="bf16 recip"):
                        nc.vector.reciprocal(out=linv_sp, in_=lsp)
                    linv2 = lpool.tile([1, HPC, QC], BF16, tag="linv2", name="linv2")
                    nc.sync.dma_start(out=linv2, in_=linv_sp)
                    for h2 in range(HPC):
                        fb = h2 * DK
                        # broadcast 1/l across the 64 head-dim partitions via
                        # a K=1 ones matmul (engines can't partition-broadcast)
                        lbps = mmps.tile([FW, QC], F32, tag="mm", name="lbps")
                        nc.tensor.matmul(
                            lbps[0:DK, :],
                            ones_b,
                            linv2[:, h2, :],
                            start=True,
                            stop=True,
                        )
                        with nc.allow_low_precision(reason="bf16 out"):
                            nc.vector.tensor_mul(
                                outT[fb : fb + DK, trow : trow + QC],
                                pvc[0:DK, h2, :],
                                lbps[0:DK, :],
                            )

            # ---- AllToAll staging: head-sharded -> token-sharded ----
            cc_in = []
            cc_out = []
            orecv = []
            for b in range(B):
                cc_in.append(dram.tile([N_CORES, FW, TPC], BF16, name=f"ccin{b}"))
                cc_out.append(dram.tile([N_CORES, FW, TPC], BF16, name=f"ccout{b}"))
                orecv.append(
                    qkpool.tile(
                        [128, N_CORES, TPC], BF16, tag=f"orecv{b}", name=f"orecv{b}"
                    )
                )

            def do_a2a(b):
                nc.gpsimd.dma_start(
                    out=cc_in[b].rearrange("p f t -> f p t"),
                    in_=outT[:, b * S : (b + 1) * S].rearrange(
                        "f (p t) -> f p t", p=N_CORES
                    ),
                )
                nc.gpsimd.collective_compute(
                    "AllToAll",
                    mybir.AluOpType.bypass,
                    replica_groups=[list(range(N_CORES))],
                    ins=[cc_in[b][:].opt()],
                    outs=[cc_out[b][:].opt()],
                )
                nc.gpsimd.dma_start(
                    out=orecv[b][:, :, :],
                    in_=cc_out[b].rearrange("p f t -> f p t"),
                )

            # ---- output projection for this core's token slice of batch b ----
            def do_proj(b):
                for tt in range(TPC // 128):
                    ysb = ypool.tile([128, D], F32, tag="y", name="ysb")
                    for ec in range(D // 512):
                        yps = mmps.tile([128, 512], F32, tag="mm", name="yps")
                        for p in range(N_CORES):
                            nc.tensor.matmul(
                                yps,
                                orecv[b][:, p, tt * 128 : (tt + 1) * 128],
                                wo_sb[p][:, ec * 512 : (ec + 1) * 512],
                                start=(p == 0),
                                stop=(p == N_CORES - 1),
                            )
                        eng = nc.vector if ec == 0 else nc.scalar
                        if ec == 0:
                            nc.vector.tensor_copy(
                                out=ysb[:, ec * 512 : (ec + 1) * 512], in_=yps
                            )
                        else:
                            nc.scalar.copy(
                                out=ysb[:, ec * 512 : (ec + 1) * 512], in_=yps
                            )
                    nc.sync.dma_start(
                        out=y[b * TPC + tt * 128 : b * TPC + (tt + 1) * 128, :],
                        in_=ysb,
                    )

            # schedule: qkv(b0) -> attn(b0) -> A2A(b0) kicked -> qkv(b1)
            # -> attn(b1) [A2A(b0) hidden under it] -> A2A(b1) -> proj(b0)
            # [overlaps A2A(b1)] -> proj(b1)
            with nc.named_scope("qkv0"):
                for ci in range(NCH // 2):
                    do_qkv_chunk(ci)
            with nc.named_scope("attn0"):
                do_attn_batch(0)
            with nc.named_scope("a2a0"):
                do_a2a(0)
            with nc.named_scope("qkv1"):
                for ci in range(NCH // 2, NCH):
                    do_qkv_chunk(ci)
            with nc.named_scope("attn1"):
                do_attn_batch(1)
            with nc.named_scope("a2a1"):
                do_a2a(1)
            with nc.named_scope("proj0"):
                do_proj(0)
            with nc.named_scope("proj1"):
                do_proj(1)

            if DEBUG_DUMP:
                dqT = nc.dram_tensor("dqT", [FW, T], BF16, kind="ExternalOutput")
                dkT = nc.dram_tensor("dkT", [FW, T], BF16, kind="ExternalOutput")
                dv4 = nc.dram_tensor(
                    "dv4", [128, T // 128, HPC, DK + 1], BF16, kind="ExternalOutput"
                )
                doutT = nc.dram_tensor("doutT", [FW, T], BF16, kind="ExternalOutput")
                dorecv = nc.dram_tensor(
                    "dorecv", [B, 128, N_CORES, TPC], BF16, kind="ExternalOutput"
                )
                nc.sync.dma_start(out=dqT[:, :], in_=qT)
                nc.sync.dma_start(out=dkT[:, :], in_=kTt)
                nc.sync.dma_start(out=dv4[:, :, :, :], in_=v4)
                nc.sync.dma_start(out=doutT[:, :], in_=outT)
                for b in range(B):
                    nc.sync.dma_start(out=dorecv[b, :, :, :], in_=orecv[b])

    _spill_waits(nc)
    return nc


_NC_CACHE = None


def _get_nc():
    global _NC_CACHE
    if _NC_CACHE is None:
        _NC_CACHE = build_kernel()
    return _NC_CACHE


def _host_prep(x, Wq, Wk, Wv, Wo, token_positions):
    bf = ml_dtypes.bfloat16
    xT = np.ascontiguousarray(x.reshape(T, D).T).astype(bf)  # [D, T]
    WqT = (np.ascontiguousarray(Wq.T) * np.float32(1.0 / math.sqrt(DK))).astype(bf)
    WkT = np.ascontiguousarray(Wk.T).astype(bf)
    WvT = np.ascontiguousarray(Wv.T).astype(bf)
    WoT = np.ascontiguousarray(Wo.T).astype(bf)

    pos = token_positions.astype(np.float64)  # [S]
    i = (np.arange(FW) % DK) // 2  # pair index per row
    inv_freq = 1.0 / (10000.0 ** (2.0 * i / DK))  # [FW]
    ang = inv_freq[:, None] * pos[None, :]  # [FW, S]
    ctab = np.cos(ang).astype(np.float32)
    sgn = np.where(np.arange(FW) % 2 == 0, -1.0, 1.0)
    stab = (np.sin(ang) * sgn[:, None]).astype(np.float32)

    trim = (np.arange(KT)[None, :] >= np.arange(KT)[:, None]).astype(bf)
    return xT, WqT, WkT, WvT, WoT, ctab, stab, trim


def kernel(x, Wq, Wk, Wv, Wo, mask, token_positions, num_heads, **run_kw):
    x = np.asarray(x)
    assert int(num_heads) == H and x.shape == (B, S, D)
    xT, WqT, WkT, WvT, WoT, ctab, stab, trim = _host_prep(
        np.asarray(x, np.float32),
        np.asarray(Wq, np.float32),
        np.asarray(Wk, np.float32),
        np.asarray(Wv, np.float32),
        np.asarray(Wo, np.float32),
        np.asarray(token_positions),
    )
    in_maps = []
    for c in range(N_CORES):
        cols = slice(c * FW, (c + 1) * FW)
        w3 = np.stack([WqT[:, cols], WkT[:, cols], WvT[:, cols]], axis=1)
        wqkv_c = np.ascontiguousarray(
            w3.reshape(8, 128, 3, FW).transpose(1, 0, 2, 3).reshape(128, 8 * 3 * FW)
        )
        in_maps.append(
            {
                "xT": xT,
                "wqkv": wqkv_c,
                "woT": WoT,
                "ctab": ctab,
                "stab": stab,
                "trim": trim,
            }
        )
    nc = _get_nc()
    res = run_bass_kernel_spmd(
        nc, in_maps, core_ids=list(range(N_CORES)), **run_kw
    )
    # core c's y rows: [0:TPC] = batch0 tokens [c*TPC,(c+1)*TPC),
    #                  [TPC:2*TPC] = batch1 same range
    out = np.empty((B, S, D), dtype=np.float32)
    for c in range(N_CORES):
        yc = res.results[c]["y"]
        for b in range(B):
            out[b, c * TPC : (c + 1) * TPC, :] = yc[b * TPC : (b + 1) * TPC]
    kernel.last_results = res
    return out
